# revision 8
# baseline (speedup 1.0000x reference)
"""Co-attention head kernel v2 for 8 Trainium2 NeuronCores.

Reference computation (H=4096, heads=4, d=1024, N=1024):
    q/k/v[h] = node1|node2 @ W{q,k,v}[h] + b        ([N, d] per head)
    r[h]     = (q[h] @ k[h]^T * 1/sqrt(d)) .* v[h]  (elementwise, N==d)
    out      = LayerNorm(concat_h r[h])             ([N, 4096])

Sharding: 8 cores = 4 heads x 2 n-halves. Core c=(h=c//2, s=c%2) computes
q/k/v for its OWN 512 n-rows over the full d=1024 (weights full per head,
activations column-sliced on host). Only kT m-halves cross cores: a pair
AllGather (1 MiB bf16) rebuilds full kT in canonical m order on both
cores, hidden under the V projection. scores = qT^T@kT land in PSUM and
are consumed there (r = scores .* v); LN stats AllReduce over same-parity
cores; each core writes out block [512, 1024].

Layouts (SBUF): qT[fb]=[128f,512n], kT halves [128f,512m], v/r=[128n,1024m].
All matmuls bf16 in / f32 psum out (full PE rate; ~3e-3 rel err).
"""

from contextlib import ExitStack

import numpy as np

import concourse.bass as bass
import concourse.tile as tile
from concourse import bacc, mybir
from concourse.bass_utils import run_bass_kernel_spmd

F32 = mybir.dt.float32
BF16 = mybir.dt.bfloat16

H_DIM = 4096
N_HEADS = 4
D_HEAD = 1024
N = 1024
HALF = 512
LN_EPS = 1e-5
N_CORES = 8
SCALE = 1.0 / 32.0  # 1/sqrt(D_HEAD)

K_TILES = H_DIM // 128  # 32
KB = 8  # k-tiles per stream DMA
G = K_TILES // KB  # 8 stream groups

ALU = mybir.AluOpType
ACT_FN = mybir.ActivationFunctionType

PAIRS = [[0, 1], [2, 3], [4, 5], [6, 7]]
QUADS = [[0, 2, 4, 6], [1, 3, 5, 7]]


def _bcast_ap(ap: bass.AP, parts: int = 128) -> bass.AP:
    """[n] DRAM vector viewed as [parts, n] with 0-stride partitions."""
    return bass.AP(tensor=ap.tensor, offset=ap.offset, ap=[[0, parts], *ap.ap])


def build_program(reps: int = 1, no_collectives: bool = False):
    nc = bacc.Bacc("TRN2", target_bir_lowering=False, debug=False, num_devices=N_CORES)

    n1s = nc.dram_tensor("n1s", [H_DIM, HALF], BF16, kind="ExternalInput").ap()
    n2s = nc.dram_tensor("n2s", [H_DIM, HALF], BF16, kind="ExternalInput").ap()
    wq = nc.dram_tensor("wq", [H_DIM, D_HEAD], BF16, kind="ExternalInput").ap()
    wk = nc.dram_tensor("wk", [H_DIM, D_HEAD], BF16, kind="ExternalInput").ap()
    wv = nc.dram_tensor("wv", [H_DIM, D_HEAD], BF16, kind="ExternalInput").ap()
    bq = nc.dram_tensor("bq", [D_HEAD], F32, kind="ExternalInput").ap()
    bk = nc.dram_tensor("bk", [D_HEAD], F32, kind="ExternalInput").ap()
    bv = nc.dram_tensor("bv", [D_HEAD], F32, kind="ExternalInput").ap()
    gam = nc.dram_tensor("gam", [D_HEAD], F32, kind="ExternalInput").ap()
    bet = nc.dram_tensor("bet", [D_HEAD], F32, kind="ExternalInput").ap()
    out = nc.dram_tensor("out", [HALF, N], BF16, kind="ExternalOutput").ap()

    # k-tile views: [128, K_TILES, free]; group a = DRAM rows a*128+p
    n1s_2 = n1s.rearrange("(a p) n -> p a n", p=128)
    n2s_2 = n2s.rearrange("(a p) n -> p a n", p=128)
    wq_2 = wq.rearrange("(a p) f -> p a f", p=128)
    wk_2 = wk.rearrange("(a p) f -> p a f", p=128)
    wv_2 = wv.rearrange("(a p) f -> p a f", p=128)

    with tile.TileContext(nc) as tc, ExitStack() as ctx:
        singles = ctx.enter_context(tc.tile_pool(name="singles", bufs=1))
        streams = ctx.enter_context(tc.tile_pool(name="streams", bufs=2))
        resident = ctx.enter_context(tc.tile_pool(name="resident", bufs=1))
        ps = ctx.enter_context(tc.tile_pool(name="ps", bufs=1, space="PSUM"))
        fin = ctx.enter_context(tc.tile_pool(name="fin", bufs=1))
        dram = ctx.enter_context(tc.tile_pool(name="dram", bufs=1, space="DRAM"))

        # ---- constants (loaded once) ----
        bq_sb = singles.tile([128, 8], F32)
        nc.sync.dma_start(out=bq_sb, in_=bq.rearrange("(b p) -> p b", p=128))
        bk_sb = singles.tile([128, 8], F32)
        nc.sync.dma_start(out=bk_sb, in_=bk.rearrange("(b p) -> p b", p=128))
        bv_b = singles.tile([128, D_HEAD], F32)
        nc.sync.dma_start(out=bv_b, in_=_bcast_ap(bv))
        gam_b = singles.tile([128, D_HEAD], F32)
        nc.sync.dma_start(out=gam_b, in_=_bcast_ap(gam))
        bet_b = singles.tile([128, D_HEAD], F32)
        nc.sync.dma_start(out=bet_b, in_=_bcast_ap(bet))
        eps_sb = singles.tile([128, 1], F32)
        nc.vector.memset(eps_sb, LN_EPS)

        def emit_rep():
            ag_in = dram.tile([N, HALF], BF16, name="ag_in", tag="ag_in", bufs=2)
            ag_out = dram.tile(
                [2 * N, HALF], BF16, name="ag_out", tag="ag_out", bufs=2
            )
            ar_in = dram.tile([HALF, 2], F32, name="ar_in", tag="ar_in", bufs=2)
            ar_out = dram.tile(
                [HALF, 2], F32, name="ar_out", tag="ar_out", bufs=2
            )

            # psum banks: 8 x [128, 512], shared by all phases
            pp = [
                ps.tile([128, 512], F32, name=f"pp{i}", tag=f"pp{i}")
                for i in range(8)
            ]
            # resident SBUF (tags shared across reps)
            qT = [
                resident.tile([128, HALF], BF16, name=f"qT{fb}", tag=f"qT{fb}")
                for fb in range(8)
            ]
            kT_own = [
                resident.tile([128, HALF], BF16, name=f"kTw{fb}", tag=f"kTw{fb}")
                for fb in range(8)
            ]
            kTh = [
                [
                    resident.tile([128, HALF], BF16, name=f"kT{m}_{fb}", tag=f"kT{m}_{fb}")
                    for fb in range(8)
                ]
                for m in range(2)
            ]
            n2s_sb = [
                resident.tile([128, KB, HALF], BF16, name=f"n2s{g}", tag=f"n2s{g}")
                for g in range(G)
            ]
            v_sb = [
                resident.tile([128, N], F32, name=f"v{nb}", tag=f"v{nb}")
                for nb in range(4)
            ]
            r_sb = [
                resident.tile([128, N], F32, name=f"r{nb}", tag=f"r{nb}")
                for nb in range(4)
            ]

            # ---- Q projection: psum[fb] = wq[:,fb]^T @ n1s ----
            for g in range(G):
                w_t = streams.tile([128, KB, D_HEAD], BF16, name="w_t", tag="w_t")
                nc.sync.dma_start(out=w_t, in_=wq_2[:, KB * g : KB * g + KB, :])
                a_t = streams.tile([128, KB, HALF], BF16, name="a_t", tag="a_t")
                nc.scalar.dma_start(out=a_t, in_=n1s_2[:, KB * g : KB * g + KB, :])
                for ah in range(KB // 4):
                    for fb in range(8):
                        for a4 in range(4):
                            a = ah * 4 + a4
                            nc.tensor.matmul(
                                pp[fb][:],
                                w_t[:, a, fb * 128 : (fb + 1) * 128],
                                a_t[:, a, :],
                                start=(g == 0 and a == 0),
                                stop=(g == G - 1 and a == KB - 1),
                            )
            for fb in range(8):
                nc.vector.tensor_scalar(
                    out=qT[fb][:],
                    in0=pp[fb][:],
                    scalar1=bq_sb[:, fb : fb + 1],
                    scalar2=None,
                    op0=ALU.add,
                )

            # ---- K projection: n2s tiles stay resident for V ----
            for g in range(G):
                w_t = streams.tile([128, KB, D_HEAD], BF16, name="w_t", tag="w_t")
                nc.sync.dma_start(out=w_t, in_=wk_2[:, KB * g : KB * g + KB, :])
                nc.scalar.dma_start(
                    out=n2s_sb[g], in_=n2s_2[:, KB * g : KB * g + KB, :]
                )
                for ah in range(KB // 4):
                    for fb in range(8):
                        for a4 in range(4):
                            a = ah * 4 + a4
                            nc.tensor.matmul(
                                pp[fb][:],
                                w_t[:, a, fb * 128 : (fb + 1) * 128],
                                n2s_sb[g][:, a, :],
                                start=(g == 0 and a == 0),
                                stop=(g == G - 1 and a == KB - 1),
                            )
            for fb in range(8):
                nc.vector.tensor_scalar(
                    out=kT_own[fb][:],
                    in0=pp[fb][:],
                    scalar1=bk_sb[:, fb : fb + 1],
                    scalar2=None,
                    op0=ALU.add,
                )

            # ---- stage kT half + pair AllGather (overlaps V projection) ----
            for fb in range(8):
                nc.gpsimd.dma_start(
                    out=ag_in[fb * 128 : (fb + 1) * 128, :], in_=kT_own[fb]
                )
            if no_collectives:
                nc.gpsimd.dma_start(out=ag_out[0:N, :], in_=ag_in[:])
                nc.gpsimd.dma_start(out=ag_out[N : 2 * N, :], in_=ag_in[:])
            else:
                nc.gpsimd.collective_compute(
                    "AllGather",
                    ALU.bypass,
                    replica_groups=PAIRS,
                    ins=[ag_in[:].opt()],
                    outs=[ag_out[:].opt()],
                )

            # ---- V projection: stationary n2s blocks, moving full wv ----
            for g in range(G):
                w_t = streams.tile([128, KB, D_HEAD], BF16, name="w_t", tag="w_t")
                nc.sync.dma_start(out=w_t, in_=wv_2[:, KB * g : KB * g + KB, :])
                for ah in range(KB // 4):
                    for nb in range(4):
                        for j in range(2):
                            for a4 in range(4):
                                a = ah * 4 + a4
                                nc.tensor.matmul(
                                    pp[nb * 2 + j][:],
                                    n2s_sb[g][:, a, nb * 128 : (nb + 1) * 128],
                                    w_t[:, a, j * 512 : (j + 1) * 512],
                                    start=(g == 0 and a == 0),
                                    stop=(g == G - 1 and a == KB - 1),
                                )
            for nb in range(4):
                for j in range(2):
                    nc.vector.tensor_add(
                        out=v_sb[nb][:, j * 512 : (j + 1) * 512],
                        in0=pp[nb * 2 + j][:],
                        in1=bv_b[:, j * 512 : (j + 1) * 512],
                    )

            # ---- read back full kT (canonical m order; both halves) ----
            for m in range(2):
                for fb in range(8):
                    nc.gpsimd.dma_start(
                        out=kTh[m][fb],
                        in_=ag_out[m * N + fb * 128 : m * N + (fb + 1) * 128, :],
                    )

            # ---- scores: psum[nb*2+mh] = qT[:,nb]^T @ kT[mh] ----
            for fbh in range(2):
                for nb in range(4):
                    for mh in range(2):
                        for f4 in range(4):
                            fb = fbh * 4 + f4
                            nc.tensor.matmul(
                                pp[nb * 2 + mh][:],
                                qT[fb][:, nb * 128 : (nb + 1) * 128],
                                kTh[mh][fb][:],
                                start=(fb == 0),
                                stop=(fb == 7),
                            )

            # ---- r = scores .* v (from PSUM), LN stats ----
            st_all = fin.tile([128, 4, 2], F32, name="st_all", tag="st_all", bufs=2)
            for nb in range(4):
                for mh in range(2):
                    nc.vector.tensor_mul(
                        out=r_sb[nb][:, mh * 512 : (mh + 1) * 512],
                        in0=pp[nb * 2 + mh][:],
                        in1=v_sb[nb][:, mh * 512 : (mh + 1) * 512],
                    )
                nc.vector.tensor_reduce(
                    out=st_all[:, nb, 0:1],
                    in_=r_sb[nb][:],
                    axis=mybir.AxisListType.X,
                    op=ALU.add,
                )
                sq_t = fin.tile([128, N], F32, name="sq_t", tag="sq_t", bufs=1)
                nc.vector.tensor_mul(out=sq_t[:], in0=r_sb[nb][:], in1=r_sb[nb][:])
                nc.vector.tensor_reduce(
                    out=st_all[:, nb, 1:2],
                    in_=sq_t[:],
                    axis=mybir.AxisListType.X,
                    op=ALU.add,
                )

            # ---- quad AllReduce of stats, normalize, write out ----
            ar_in_2 = ar_in[:].rearrange("(b p) c -> p b c", p=128)
            ar_out_2 = ar_out[:].rearrange("(b p) c -> p b c", p=128)
            nc.gpsimd.dma_start(out=ar_in_2, in_=st_all)
            nc.gpsimd.collective_compute(
                "AllReduce",
                ALU.add,
                replica_groups=QUADS,
                ins=[ar_in[:].opt()],
                outs=[ar_out[:].opt()],
            )
            tot_all = fin.tile([128, 4, 2], F32, name="tot_all", tag="tot_all", bufs=2)
            nc.gpsimd.dma_start(out=tot_all, in_=ar_out_2)
            inv_h = 1.0 / float(H_DIM)
            for nb in range(4):
                mu_t = fin.tile([128, 1], F32, name=f"mu{nb}", tag=f"mu{nb}")
                nc.vector.tensor_scalar_mul(
                    out=mu_t, in0=tot_all[:, nb, 0:1], scalar1=inv_h
                )
                msq_t = fin.tile([128, 1], F32, name=f"msq{nb}", tag=f"msq{nb}")
                nc.vector.tensor_mul(out=msq_t, in0=mu_t, in1=mu_t)
                var_t = fin.tile([128, 1], F32, name=f"var{nb}", tag=f"var{nb}")
                nc.vector.tensor_scalar(
                    out=var_t,
                    in0=tot_all[:, nb, 1:2],
                    scalar1=inv_h,
                    scalar2=msq_t[:, 0:1],
                    op0=ALU.mult,
                    op1=ALU.subtract,
                )
                nc.scalar.activation(
                    out=var_t, in_=var_t, func=ACT_FN.Sqrt, bias=eps_sb[:], scale=1.0
                )
                nc.vector.reciprocal(out=var_t, in_=var_t)
                o_t = fin.tile([128, N], F32, name="o_t", tag="o_t", bufs=2)[:]
                nc.vector.tensor_scalar(
                    out=o_t,
                    in0=r_sb[nb][:],
                    scalar1=mu_t[:, 0:1],
                    scalar2=var_t[:, 0:1],
                    op0=ALU.subtract,
                    op1=ALU.mult,
                )
                nc.vector.tensor_mul(out=o_t, in0=o_t, in1=gam_b[:])
                o_b = fin.tile([128, N], BF16, name="o_b", tag="o_b", bufs=2)[:]
                nc.vector.tensor_add(out=o_b, in0=o_t, in1=bet_b[:])
                nc.sync.dma_start(out=out[nb * 128 : (nb + 1) * 128, :], in_=o_b)

        for _ in range(reps):
            emit_rep()

    nc.compile()
    return nc


_NC = None


def _get_program():
    global _NC
    if _NC is None:
        _NC = build_program()
    return _NC


def make_in_maps(node1, node2, Wq, bq, Wk, bk, Wv, bv, gamma, beta):
    import ml_dtypes

    f32 = np.float32
    wd = ml_dtypes.bfloat16
    n1t = np.asarray(node1, dtype=f32).T
    n2t = np.asarray(node2, dtype=f32).T
    in_maps = []
    for c in range(N_CORES):
        h, s = c // 2, c % 2
        nsl = slice(s * HALF, (s + 1) * HALF)
        in_maps.append(
            {
                "n1s": np.ascontiguousarray(n1t[:, nsl]).astype(wd),
                "n2s": np.ascontiguousarray(n2t[:, nsl]).astype(wd),
                "wq": np.ascontiguousarray(Wq[h] * SCALE).astype(wd),
                "wk": np.ascontiguousarray(Wk[h]).astype(wd),
                "wv": np.ascontiguousarray(Wv[h]).astype(wd),
                "bq": np.ascontiguousarray(bq[h] * SCALE, dtype=f32),
                "bk": np.ascontiguousarray(bk[h], dtype=f32),
                "bv": np.ascontiguousarray(bv[h], dtype=f32),
                "gam": np.ascontiguousarray(gamma[h * 1024 : (h + 1) * 1024], dtype=f32),
                "bet": np.ascontiguousarray(beta[h * 1024 : (h + 1) * 1024], dtype=f32),
            }
        )
    return in_maps


def assemble(results):
    out = np.empty((N, H_DIM), np.float32)
    for c in range(N_CORES):
        h, s = c // 2, c % 2
        out[s * HALF : (s + 1) * HALF, h * 1024 : (h + 1) * 1024] = np.asarray(
            results[c]["out"], dtype=np.float32
        )
    return out


def kernel(node1, node2, Wq, bq, Wk, bk, Wv, bv, gamma, beta):
    nc = _get_program()
    in_maps = make_in_maps(node1, node2, Wq, bq, Wk, bk, Wv, bv, gamma, beta)
    res = run_bass_kernel_spmd(nc, in_maps, list(range(N_CORES)))
    return assemble(res.results)


# revision 9
# speedup vs baseline: 1.0288x; 1.0288x over previous
"""Co-attention head kernel v2 for 8 Trainium2 NeuronCores.

Reference computation (H=4096, heads=4, d=1024, N=1024):
    q/k/v[h] = node1|node2 @ W{q,k,v}[h] + b        ([N, d] per head)
    r[h]     = (q[h] @ k[h]^T * 1/sqrt(d)) .* v[h]  (elementwise, N==d)
    out      = LayerNorm(concat_h r[h])             ([N, 4096])

Sharding: 8 cores = 4 heads x 2 n-halves. Core c=(h=c//2, s=c%2) computes
q/k/v for its OWN 512 n-rows over the full d=1024 (weights full per head,
activations column-sliced on host). Only kT m-halves cross cores: a pair
AllGather (1 MiB bf16) rebuilds full kT in canonical m order on both
cores, hidden under the V projection. scores = qT^T@kT land in PSUM and
are consumed there (r = scores .* v); LN stats AllReduce over same-parity
cores; each core writes out block [512, 1024].

Layouts (SBUF): qT[fb]=[128f,512n], kT halves [128f,512m], v/r=[128n,1024m].
All matmuls bf16 in / f32 psum out (full PE rate; ~3e-3 rel err).
"""

from contextlib import ExitStack

import numpy as np

import concourse.bass as bass
import concourse.tile as tile
from concourse import bacc, mybir
from concourse.bass_utils import run_bass_kernel_spmd

F32 = mybir.dt.float32
BF16 = mybir.dt.bfloat16

H_DIM = 4096
N_HEADS = 4
D_HEAD = 1024
N = 1024
HALF = 512
LN_EPS = 1e-5
N_CORES = 8
SCALE = 1.0 / 32.0  # 1/sqrt(D_HEAD)

K_TILES = H_DIM // 128  # 32
KB = 4  # k-tiles per stream DMA
G = K_TILES // KB  # 8 stream groups

ALU = mybir.AluOpType
ACT_FN = mybir.ActivationFunctionType

PAIRS = [[0, 1], [2, 3], [4, 5], [6, 7]]
QUADS = [[0, 2, 4, 6], [1, 3, 5, 7]]


def _bcast_ap(ap: bass.AP, parts: int = 128) -> bass.AP:
    """[n] DRAM vector viewed as [parts, n] with 0-stride partitions."""
    return bass.AP(tensor=ap.tensor, offset=ap.offset, ap=[[0, parts], *ap.ap])


def build_program(reps: int = 1, no_collectives: bool = False):
    nc = bacc.Bacc("TRN2", target_bir_lowering=False, debug=False, num_devices=N_CORES)

    n1s = nc.dram_tensor("n1s", [H_DIM, HALF], BF16, kind="ExternalInput").ap()
    n2s = nc.dram_tensor("n2s", [H_DIM, HALF], BF16, kind="ExternalInput").ap()
    wq = nc.dram_tensor("wq", [H_DIM, D_HEAD], BF16, kind="ExternalInput").ap()
    wk = nc.dram_tensor("wk", [H_DIM, D_HEAD], BF16, kind="ExternalInput").ap()
    wv = nc.dram_tensor("wv", [H_DIM, D_HEAD], BF16, kind="ExternalInput").ap()
    bq = nc.dram_tensor("bq", [D_HEAD], F32, kind="ExternalInput").ap()
    bk = nc.dram_tensor("bk", [D_HEAD], F32, kind="ExternalInput").ap()
    bv = nc.dram_tensor("bv", [D_HEAD], F32, kind="ExternalInput").ap()
    gam = nc.dram_tensor("gam", [D_HEAD], F32, kind="ExternalInput").ap()
    bet = nc.dram_tensor("bet", [D_HEAD], F32, kind="ExternalInput").ap()
    out = nc.dram_tensor("out", [HALF, N], BF16, kind="ExternalOutput").ap()

    # k-tile views: [128, K_TILES, free]; group a = DRAM rows a*128+p
    n1s_2 = n1s.rearrange("(a p) n -> p a n", p=128)
    n2s_2 = n2s.rearrange("(a p) n -> p a n", p=128)
    wq_2 = wq.rearrange("(a p) f -> p a f", p=128)
    wk_2 = wk.rearrange("(a p) f -> p a f", p=128)
    wv_2 = wv.rearrange("(a p) f -> p a f", p=128)

    with tile.TileContext(nc) as tc, ExitStack() as ctx:
        singles = ctx.enter_context(tc.tile_pool(name="singles", bufs=1))
        streams = ctx.enter_context(tc.tile_pool(name="streams", bufs=3))
        resident = ctx.enter_context(tc.tile_pool(name="resident", bufs=1))
        ps = ctx.enter_context(tc.tile_pool(name="ps", bufs=1, space="PSUM"))
        fin = ctx.enter_context(tc.tile_pool(name="fin", bufs=1))
        dram = ctx.enter_context(tc.tile_pool(name="dram", bufs=1, space="DRAM"))

        # ---- constants (loaded once) ----
        bq_sb = singles.tile([128, 8], F32)
        nc.sync.dma_start(out=bq_sb, in_=bq.rearrange("(b p) -> p b", p=128))
        bk_sb = singles.tile([128, 8], F32)
        nc.sync.dma_start(out=bk_sb, in_=bk.rearrange("(b p) -> p b", p=128))
        bv_b = singles.tile([128, D_HEAD], F32)
        nc.sync.dma_start(out=bv_b, in_=_bcast_ap(bv))
        gam_b = singles.tile([128, D_HEAD], F32)
        nc.sync.dma_start(out=gam_b, in_=_bcast_ap(gam))
        bet_b = singles.tile([128, D_HEAD], F32)
        nc.sync.dma_start(out=bet_b, in_=_bcast_ap(bet))
        eps_sb = singles.tile([128, 1], F32)
        nc.vector.memset(eps_sb, LN_EPS)

        def emit_rep():
            ag_in = dram.tile([N, HALF], BF16, name="ag_in", tag="ag_in", bufs=2)
            ag_out = dram.tile(
                [2 * N, HALF], BF16, name="ag_out", tag="ag_out", bufs=2
            )
            ar_in = dram.tile([HALF, 2], F32, name="ar_in", tag="ar_in", bufs=2)
            ar_out = dram.tile(
                [HALF, 2], F32, name="ar_out", tag="ar_out", bufs=2
            )

            # psum banks: 8 x [128, 512], shared by all phases
            pp = [
                ps.tile([128, 512], F32, name=f"pp{i}", tag=f"pp{i}")
                for i in range(8)
            ]
            # resident SBUF (tags shared across reps)
            qT = [
                resident.tile([128, HALF], BF16, name=f"qT{fb}", tag=f"qT{fb}")
                for fb in range(8)
            ]
            kT_own = [
                resident.tile([128, HALF], BF16, name=f"kTw{fb}", tag=f"kTw{fb}")
                for fb in range(8)
            ]
            kTh = [
                [
                    resident.tile([128, HALF], BF16, name=f"kT{m}_{fb}", tag=f"kT{m}_{fb}")
                    for fb in range(8)
                ]
                for m in range(2)
            ]
            n2s_sb = [
                resident.tile([128, KB, HALF], BF16, name=f"n2s{g}", tag=f"n2s{g}")
                for g in range(G)
            ]
            v_sb = [
                resident.tile([128, N], F32, name=f"v{nb}", tag=f"v{nb}")
                for nb in range(4)
            ]
            r_sb = [
                resident.tile([128, N], F32, name=f"r{nb}", tag=f"r{nb}")
                for nb in range(4)
            ]

            # ---- Q projection: psum[fb] = wq[:,fb]^T @ n1s ----
            for g in range(G):
                w_t = streams.tile([128, KB, D_HEAD], BF16, name="w_t", tag="w_t")
                nc.sync.dma_start(out=w_t, in_=wq_2[:, KB * g : KB * g + KB, :])
                a_t = streams.tile([128, KB, HALF], BF16, name="a_t", tag="a_t")
                nc.scalar.dma_start(out=a_t, in_=n1s_2[:, KB * g : KB * g + KB, :])
                for ah in range(KB // 4):
                    for fb in range(8):
                        for a4 in range(4):
                            a = ah * 4 + a4
                            nc.tensor.matmul(
                                pp[fb][:],
                                w_t[:, a, fb * 128 : (fb + 1) * 128],
                                a_t[:, a, :],
                                start=(g == 0 and a == 0),
                                stop=(g == G - 1 and a == KB - 1),
                            )
            for fb in range(8):
                nc.vector.tensor_scalar(
                    out=qT[fb][:],
                    in0=pp[fb][:],
                    scalar1=bq_sb[:, fb : fb + 1],
                    scalar2=None,
                    op0=ALU.add,
                )

            # ---- K projection: n2s tiles stay resident for V ----
            for g in range(G):
                w_t = streams.tile([128, KB, D_HEAD], BF16, name="w_t", tag="w_t")
                nc.sync.dma_start(out=w_t, in_=wk_2[:, KB * g : KB * g + KB, :])
                nc.scalar.dma_start(
                    out=n2s_sb[g], in_=n2s_2[:, KB * g : KB * g + KB, :]
                )
                for ah in range(KB // 4):
                    for fb in range(8):
                        for a4 in range(4):
                            a = ah * 4 + a4
                            nc.tensor.matmul(
                                pp[fb][:],
                                w_t[:, a, fb * 128 : (fb + 1) * 128],
                                n2s_sb[g][:, a, :],
                                start=(g == 0 and a == 0),
                                stop=(g == G - 1 and a == KB - 1),
                            )
            for fb in range(8):
                nc.vector.tensor_scalar(
                    out=kT_own[fb][:],
                    in0=pp[fb][:],
                    scalar1=bk_sb[:, fb : fb + 1],
                    scalar2=None,
                    op0=ALU.add,
                )

            # ---- stage kT half + pair AllGather (overlaps V projection) ----
            for fb in range(8):
                nc.gpsimd.dma_start(
                    out=ag_in[fb * 128 : (fb + 1) * 128, :], in_=kT_own[fb]
                )
            if no_collectives:
                nc.gpsimd.dma_start(out=ag_out[0:N, :], in_=ag_in[:])
                nc.gpsimd.dma_start(out=ag_out[N : 2 * N, :], in_=ag_in[:])
            else:
                nc.gpsimd.collective_compute(
                    "AllGather",
                    ALU.bypass,
                    replica_groups=PAIRS,
                    ins=[ag_in[:].opt()],
                    outs=[ag_out[:].opt()],
                )

            # ---- V projection: stationary n2s blocks, moving full wv ----
            for g in range(G):
                w_t = streams.tile([128, KB, D_HEAD], BF16, name="w_t", tag="w_t")
                nc.sync.dma_start(out=w_t, in_=wv_2[:, KB * g : KB * g + KB, :])
                for ah in range(KB // 4):
                    for nb in range(4):
                        for j in range(2):
                            for a4 in range(4):
                                a = ah * 4 + a4
                                nc.tensor.matmul(
                                    pp[nb * 2 + j][:],
                                    n2s_sb[g][:, a, nb * 128 : (nb + 1) * 128],
                                    w_t[:, a, j * 512 : (j + 1) * 512],
                                    start=(g == 0 and a == 0),
                                    stop=(g == G - 1 and a == KB - 1),
                                )
            for nb in range(4):
                for j in range(2):
                    nc.vector.tensor_add(
                        out=v_sb[nb][:, j * 512 : (j + 1) * 512],
                        in0=pp[nb * 2 + j][:],
                        in1=bv_b[:, j * 512 : (j + 1) * 512],
                    )

            # ---- read back full kT (canonical m order; both halves) ----
            for m in range(2):
                for fb in range(8):
                    nc.gpsimd.dma_start(
                        out=kTh[m][fb],
                        in_=ag_out[m * N + fb * 128 : m * N + (fb + 1) * 128, :],
                    )

            # ---- scores: psum[nb*2+mh] = qT[:,nb]^T @ kT[mh] ----
            for nb in range(4):
                for fbh in range(2):
                    for mh in range(2):
                        for f4 in range(4):
                            fb = fbh * 4 + f4
                            nc.tensor.matmul(
                                pp[nb * 2 + mh][:],
                                qT[fb][:, nb * 128 : (nb + 1) * 128],
                                kTh[mh][fb][:],
                                start=(fb == 0),
                                stop=(fb == 7),
                            )

            # ---- r = scores .* v (from PSUM), LN stats ----
            st_all = fin.tile([128, 4, 2], F32, name="st_all", tag="st_all", bufs=2)
            for nb in range(4):
                for mh in range(2):
                    nc.vector.tensor_mul(
                        out=r_sb[nb][:, mh * 512 : (mh + 1) * 512],
                        in0=pp[nb * 2 + mh][:],
                        in1=v_sb[nb][:, mh * 512 : (mh + 1) * 512],
                    )
                nc.vector.tensor_reduce(
                    out=st_all[:, nb, 0:1],
                    in_=r_sb[nb][:],
                    axis=mybir.AxisListType.X,
                    op=ALU.add,
                )
                sq_t = fin.tile([128, N], F32, name="sq_t", tag="sq_t", bufs=1)
                nc.vector.tensor_mul(out=sq_t[:], in0=r_sb[nb][:], in1=r_sb[nb][:])
                nc.vector.tensor_reduce(
                    out=st_all[:, nb, 1:2],
                    in_=sq_t[:],
                    axis=mybir.AxisListType.X,
                    op=ALU.add,
                )

            # ---- quad AllReduce of stats, normalize, write out ----
            ar_in_2 = ar_in[:].rearrange("(b p) c -> p b c", p=128)
            ar_out_2 = ar_out[:].rearrange("(b p) c -> p b c", p=128)
            nc.gpsimd.dma_start(out=ar_in_2, in_=st_all)
            nc.gpsimd.collective_compute(
                "AllReduce",
                ALU.add,
                replica_groups=QUADS,
                ins=[ar_in[:].opt()],
                outs=[ar_out[:].opt()],
            )
            tot_all = fin.tile([128, 4, 2], F32, name="tot_all", tag="tot_all", bufs=2)
            nc.gpsimd.dma_start(out=tot_all, in_=ar_out_2)
            inv_h = 1.0 / float(H_DIM)
            for nb in range(4):
                mu_t = fin.tile([128, 1], F32, name=f"mu{nb}", tag=f"mu{nb}")
                nc.vector.tensor_scalar_mul(
                    out=mu_t, in0=tot_all[:, nb, 0:1], scalar1=inv_h
                )
                msq_t = fin.tile([128, 1], F32, name=f"msq{nb}", tag=f"msq{nb}")
                nc.vector.tensor_mul(out=msq_t, in0=mu_t, in1=mu_t)
                var_t = fin.tile([128, 1], F32, name=f"var{nb}", tag=f"var{nb}")
                nc.vector.tensor_scalar(
                    out=var_t,
                    in0=tot_all[:, nb, 1:2],
                    scalar1=inv_h,
                    scalar2=msq_t[:, 0:1],
                    op0=ALU.mult,
                    op1=ALU.subtract,
                )
                nc.scalar.activation(
                    out=var_t, in_=var_t, func=ACT_FN.Sqrt, bias=eps_sb[:], scale=1.0
                )
                nc.vector.reciprocal(out=var_t, in_=var_t)
                o_t = fin.tile([128, N], F32, name="o_t", tag="o_t", bufs=2)[:]
                nc.vector.tensor_scalar(
                    out=o_t,
                    in0=r_sb[nb][:],
                    scalar1=mu_t[:, 0:1],
                    scalar2=var_t[:, 0:1],
                    op0=ALU.subtract,
                    op1=ALU.mult,
                )
                nc.vector.tensor_mul(out=o_t, in0=o_t, in1=gam_b[:])
                o_b = fin.tile([128, N], BF16, name="o_b", tag="o_b", bufs=2)[:]
                nc.vector.tensor_add(out=o_b, in0=o_t, in1=bet_b[:])
                nc.sync.dma_start(out=out[nb * 128 : (nb + 1) * 128, :], in_=o_b)

        for _ in range(reps):
            emit_rep()

    nc.compile()
    return nc


_NC = None


def _get_program():
    global _NC
    if _NC is None:
        _NC = build_program()
    return _NC


def make_in_maps(node1, node2, Wq, bq, Wk, bk, Wv, bv, gamma, beta):
    import ml_dtypes

    f32 = np.float32
    wd = ml_dtypes.bfloat16
    n1t = np.asarray(node1, dtype=f32).T
    n2t = np.asarray(node2, dtype=f32).T
    in_maps = []
    for c in range(N_CORES):
        h, s = c // 2, c % 2
        nsl = slice(s * HALF, (s + 1) * HALF)
        in_maps.append(
            {
                "n1s": np.ascontiguousarray(n1t[:, nsl]).astype(wd),
                "n2s": np.ascontiguousarray(n2t[:, nsl]).astype(wd),
                "wq": np.ascontiguousarray(Wq[h] * SCALE).astype(wd),
                "wk": np.ascontiguousarray(Wk[h]).astype(wd),
                "wv": np.ascontiguousarray(Wv[h]).astype(wd),
                "bq": np.ascontiguousarray(bq[h] * SCALE, dtype=f32),
                "bk": np.ascontiguousarray(bk[h], dtype=f32),
                "bv": np.ascontiguousarray(bv[h], dtype=f32),
                "gam": np.ascontiguousarray(gamma[h * 1024 : (h + 1) * 1024], dtype=f32),
                "bet": np.ascontiguousarray(beta[h * 1024 : (h + 1) * 1024], dtype=f32),
            }
        )
    return in_maps


def assemble(results):
    out = np.empty((N, H_DIM), np.float32)
    for c in range(N_CORES):
        h, s = c // 2, c % 2
        out[s * HALF : (s + 1) * HALF, h * 1024 : (h + 1) * 1024] = np.asarray(
            results[c]["out"], dtype=np.float32
        )
    return out


def kernel(node1, node2, Wq, bq, Wk, bk, Wv, bv, gamma, beta):
    nc = _get_program()
    in_maps = make_in_maps(node1, node2, Wq, bq, Wk, bk, Wv, bv, gamma, beta)
    res = run_bass_kernel_spmd(nc, in_maps, list(range(N_CORES)))
    return assemble(res.results)
